# revision 7
# baseline (speedup 1.0000x reference)
"""Custom LSTM-cell kernel for Trainium2, data-parallel over batch on 8 NeuronCores.

Math (per token, elementwise over dff except the two GEMMs):
    gates = Hi @ Wh + Zi @ Wz + bias         # [tok, 4*dff], gate order I|F|O|Z
    A   = F~ + Mi
    M_t = max(A, I~) = A - min(A - I~, 0)
    F_t = exp(min(A - I~, 0))
    I_t = exp(min(I~ - A, 0))
    O_t = sigmoid(O~) = 0.5*(1 + tanh(O~/2))
    Z_t = tanh(Z~)
    N_t = F_t*Ni + I_t
    C_t = (Ci*F_t + Z_t*I_t)*m + (1-m)*Ci
    H_t = O_t*(C_t/N_t)*m + (1-m)*Hi

GEMM strategy: compensated fp8 with DoubleRow perf mode (0.5 PE cycles per
output column, 2x the fp16 rate; each DR matmul contracts a 256-row k-pair).
Host splits X = [Hi|Zi]*sX and W*sW into hi + lo fp8(e4m3) parts, where
lo = Q8(exact - hi) rides fp8's own exponent (no separate scale). Per gate a
mode picks the accumulated terms: 3 = hi*hi + lo*hi + hi*lo (near-fp16
accuracy, 18 DR matmuls), 2 = hi*hi + lo*hi (X-compensated, 12), 1 = plain
hi*hi (6). I/F gates feed exp() so they use mode 3; Z drives C_t (mode 3);
O only enters H_t through a sigmoid with 1/4 slope (mode 1 suffices).
All products carry scale s = sX*sW, divided out during PSUM readout (free:
folded into scalar_tensor_tensor / activation-scale operands).

Biases: F-gate bias is folded into Mi on the host (Mi + bF - bI); the I-gate
bias cancels inside F_t/I_t and is added to M_t on the host after gather;
O/Z biases (x s) are seeded into PSUM by K=1 fp16 ones-row matmuls.

Elementwise: tokens on partitions, dff on the free dim, all tensors fp16 in
SBUF (DVE 2x/4x perf modes) with fp32 PSUM reads; outputs DMA'd fp16 and
upcast on the host. Engine split keeps every engine under the PE time:
ScalarE: I-gate PSUM copy, 2x tanh (from PSUM, scale-folded), 2x exp.
DVE: A, Dd, min-clamps, M_t, reciprocal, Ct, and the H_t chain.
GPSIMD: N_t chain and the two remaining multiplies.
"""

import numpy as np
import ml_dtypes

import concourse.bass as bass
import concourse.tile as tile
import concourse.bass_utils as bass_utils
from concourse import bacc, mybir
from concourse.bass import ts, ds

B, P, D, DFF = 256, 64, 512, 1024
NCORES = 8
BL = B // NCORES          # batches per core
TOK = BL * P              # tokens per core (2048)
NT = TOK // 128           # token tiles per core (16)
KT = (DFF + D) // 128     # k-tiles over X=[Hi|Zi] (12)
KK = KT // 2              # DoubleRow k-pairs (6)
CH = 2                    # dff column chunks of 512 per gate
CW = 512                  # chunk width
SX = 32.0                 # fp8 scale for X (e4m3 max 240; X outliers ~6 sigma)
SW = 4096.0               # fp8 scale for W (|W|max ~0.0442 -> 181)
SINV = 1.0 / (SX * SW)
import os
MODES = tuple(int(c) for c in os.environ.get("K_MODES", "3333"))
assert len(MODES) == 4    # compensation mode per gate I, F, O, Z
G3 = [g for g in range(4) if MODES[g] >= 3]
G2 = [g for g in range(4) if MODES[g] >= 2]

F32 = mybir.dt.float32
F16 = mybir.dt.float16
F8 = mybir.dt.float8e4
AF = mybir.ActivationFunctionType
OP = mybir.AluOpType
DR = mybir.MatmulPerfMode.DoubleRow

_CACHE = {}


def _build(repeat: int = 1):
    key = repeat
    if key in _CACHE:
        return _CACHE[key]

    nc = bacc.Bacc("TRN2", target_bir_lowering=False, debug=False,
                   num_devices=NCORES)

    xhiT = nc.dram_tensor("xhiT", [KK, 2, 128, TOK], F8, kind="ExternalInput").ap()
    xloT = nc.dram_tensor("xloT", [KK, 2, 128, TOK], F8, kind="ExternalInput").ap()
    whi = nc.dram_tensor("whi", [CH, KK, 128, 2, 4, CW], F8,
                         kind="ExternalInput").ap()
    wlo = nc.dram_tensor("wlo", [CH, KK, 128, 2, len(G3), CW], F8,
                         kind="ExternalInput").ap()
    boz = nc.dram_tensor("boz", [1, CH, 2, CW], F16, kind="ExternalInput").ap()
    mi = nc.dram_tensor("mi", [TOK, DFF], F16, kind="ExternalInput").ap()
    ci = nc.dram_tensor("ci", [TOK, DFF], F16, kind="ExternalInput").ap()
    ni = nc.dram_tensor("ni", [TOK, DFF], F16, kind="ExternalInput").ap()
    hiom = nc.dram_tensor("hiom", [TOK, DFF], F16, kind="ExternalInput").ap()
    mpk = nc.dram_tensor("mpk", [NT, 128, 3], F32, kind="ExternalInput").ap()

    ct = nc.dram_tensor("ct", [TOK, DFF], F16, kind="ExternalOutput").ap()
    mt = nc.dram_tensor("mt", [TOK, DFF], F16, kind="ExternalOutput").ap()
    ht = nc.dram_tensor("ht", [TOK, DFF], F16, kind="ExternalOutput").ap()
    nt = nc.dram_tensor("nt", [TOK, DFF], F16, kind="ExternalOutput").ap()

    with tile.TileContext(nc) as tc:
        with (
            tc.tile_pool(name="singles", bufs=1) as singles,
            tc.tile_pool(name="wpool", bufs=KK + 2) as wpool,
            tc.tile_pool(name="inpool", bufs=2) as inpool,
            tc.tile_pool(name="tmpA", bufs=1) as tmpA,
            tc.tile_pool(name="tmpB", bufs=2) as tmpB,
            tc.tile_pool(name="outp", bufs=2) as outp,
            tc.tile_pool(name="ps", bufs=8, space="PSUM") as pspool,
        ):
            xhi_sb = singles.tile([128, KK, 2, TOK], F8)
            xlo_sb = singles.tile([128, KK, 2, TOK], F8)
            for kk in range(KK):
                for j in range(2):
                    nc.sync.dma_start(out=xhi_sb[:, kk, j], in_=xhiT[kk, j])
                    nc.sync.dma_start(out=xlo_sb[:, kk, j], in_=xloT[kk, j])
            mpk_sb = singles.tile([128, NT, 3], F32)
            nc.sync.dma_start(out=mpk_sb, in_=mpk.rearrange("t p c -> p t c"))
            boz_sb = singles.tile([1, CH, 2, CW], F16)
            nc.sync.dma_start(out=boz_sb, in_=boz)
            ones_sb = singles.tile([1, 128], F16)
            nc.vector.memset(ones_sb, 1.0)

            for _ in range(repeat):
                for c in range(CH):
                    whik, wlok = [], []
                    for kk in range(KK):
                        wt = wpool.tile([128, 2, 4, CW], F8, tag="whi")
                        nc.sync.dma_start(out=wt, in_=whi[c, kk])
                        whik.append(wt)
                        wl = wpool.tile([128, 2, len(G3), CW], F8, tag="wlo")
                        nc.sync.dma_start(out=wl, in_=wlo[c, kk])
                        wlok.append(wl)
                    for t in range(NT):
                        rows = ts(t, 128)
                        cols = ds(c * CW, CW)
                        mi_t = inpool.tile([128, CW], F16, tag="mi")
                        nc.sync.dma_start(out=mi_t, in_=mi[rows, cols])
                        ci_t = inpool.tile([128, CW], F16, tag="ci")
                        nc.sync.dma_start(out=ci_t, in_=ci[rows, cols])
                        ni_t = inpool.tile([128, CW], F16, tag="ni")
                        nc.sync.dma_start(out=ni_t, in_=ni[rows, cols])
                        ho_t = inpool.tile([128, CW], F16, tag="ho")
                        nc.sync.dma_start(out=ho_t, in_=hiom[rows, cols])
                        m_ap = mpk_sb[:, t, 0:1]
                        om_ap = mpk_sb[:, t, 1:2]
                        hm_ap = mpk_sb[:, t, 2:3]

                        ps = [pspool.tile([128, CW], F32, tag="ps",
                                          name=f"ps{g}") for g in range(4)]
                        # O/Z bias (x s) seeded via K=1 fp16 ones-row matmul
                        for gi, g in enumerate((2, 3)):
                            nc.tensor.matmul(ps[g], ones_sb,
                                             boz_sb[0:1, c, gi],
                                             start=True, stop=True)
                        # last matmul index per gate for stop flags
                        nmm = [6 * (1 + (MODES[g] >= 2) + (MODES[g] >= 3))
                               for g in range(4)]
                        cnt = [0] * 4

                        def mm(g, lhsT, rhs):
                            cnt[g] += 1
                            nc.tensor.matmul(
                                ps[g], lhsT, rhs,
                                start=(g < 2 and cnt[g] == 1),
                                stop=(cnt[g] == nmm[g]),
                                perf_mode=DR, skip_group_check=True)

                        for kk in range(KK):
                            xh = xhi_sb[:, kk, :, rows]
                            for g in range(4):
                                mm(g, xh, whik[kk][:, :, g])
                            for gi, g in enumerate(G3):
                                mm(g, xh, wlok[kk][:, :, gi])
                        for kk in range(KK):
                            xl = xlo_sb[:, kk, :, rows]
                            for g in G2:
                                mm(g, xl, whik[kk][:, :, g])

                        psI, psF, psO, psZ = ps
                        # PSUM readout (fp32) -> fp16 SBUF working set
                        tmpI = tmpB.tile([128, CW], F16, tag="tmpI")
                        nc.scalar.activation(tmpI, psI, AF.Copy, scale=SINV)
                        A = tmpA.tile([128, CW], F16, tag="A")
                        nc.vector.scalar_tensor_tensor(A, psF, SINV, mi_t,
                                                       OP.mult, OP.add)
                        th = tmpB.tile([128, CW], F16, tag="th")
                        nc.scalar.activation(th, psO, AF.Tanh, scale=0.5 * SINV)
                        Zt = tmpB.tile([128, CW], F16, tag="Zt")
                        nc.scalar.activation(Zt, psZ, AF.Tanh, scale=SINV)

                        Dd = tmpA.tile([128, CW], F16, tag="Dd")
                        nc.vector.tensor_sub(Dd, A, tmpI)
                        p_ = tmpA.tile([128, CW], F16, tag="p")
                        nc.vector.tensor_scalar_min(p_, Dd, 0.0)
                        pn = tmpA.tile([128, CW], F16, tag="pn")
                        nc.vector.tensor_scalar(pn, Dd, -1.0, 0.0, OP.mult,
                                                OP.min)
                        Mt = outp.tile([128, CW], F16, tag="Mt")
                        nc.vector.tensor_sub(Mt, A, p_)
                        Ft = tmpB.tile([128, CW], F16, tag="Ft")
                        nc.scalar.activation(Ft, p_, AF.Exp)
                        It = tmpB.tile([128, CW], F16, tag="It")
                        nc.scalar.activation(It, pn, AF.Exp)

                        FN = tmpA.tile([128, CW], F16, tag="FN")
                        nc.gpsimd.tensor_mul(FN, Ft, ni_t)
                        NtF = tmpA.tile([128, CW], F32, tag="NtF")
                        nc.gpsimd.tensor_add(NtF, FN, It)
                        Nt = outp.tile([128, CW], F16, tag="Nt")
                        nc.scalar.activation(Nt, NtF, AF.Copy, scale=1.0)
                        rec = tmpB.tile([128, CW], F32, tag="rec")
                        nc.vector.reciprocal_approx_fast(rec, NtF)

                        mF = tmpA.tile([128, CW], F16, tag="mF")
                        nc.vector.tensor_scalar(mF, Ft, m_ap, om_ap, OP.mult,
                                                OP.add)
                        p1 = tmpA.tile([128, CW], F16, tag="p1")
                        nc.gpsimd.tensor_mul(p1, ci_t, mF)
                        t2 = tmpA.tile([128, CW], F16, tag="t2")
                        nc.gpsimd.tensor_mul(t2, Zt, It)
                        Ct = outp.tile([128, CW], F16, tag="Ct")
                        nc.vector.scalar_tensor_tensor(Ct, t2, m_ap, p1,
                                                       OP.mult, OP.add)

                        thp = tmpA.tile([128, CW], F16, tag="thp")
                        nc.vector.tensor_scalar(thp, th, hm_ap, hm_ap,
                                                OP.mult, OP.add)
                        x1 = tmpA.tile([128, CW], F16, tag="x1")
                        nc.vector.tensor_mul(x1, Ct, rec)
                        x2 = tmpA.tile([128, CW], F16, tag="x2")
                        nc.vector.tensor_mul(x2, x1, thp)
                        Ht = outp.tile([128, CW], F16, tag="Ht")
                        nc.vector.tensor_add(Ht, x2, ho_t)

                        nc.sync.dma_start(out=mt[rows, cols], in_=Mt)
                        nc.sync.dma_start(out=nt[rows, cols], in_=Nt)
                        nc.sync.dma_start(out=ct[rows, cols], in_=Ct)
                        nc.sync.dma_start(out=ht[rows, cols], in_=Ht)

    nc.compile()
    _CACHE[key] = nc
    return nc


def _q8(x):
    return x.astype(ml_dtypes.float8_e4m3)


def _prep_inputs(inputs):
    """Host-side shard + reformat. Returns per-core input maps."""
    f32, f16 = np.float32, np.float16
    g = {k: np.asarray(v) for k, v in inputs.items()}

    Wh = np.concatenate([g['WI_w'], g['WF_w'], g['WO_w'], g['WZ_w']], axis=1)
    Wz = np.concatenate([g['RI_w'], g['RF_w'], g['RO_w'], g['RZ_w']], axis=1)
    bias = np.concatenate([g['WI_b'] + g['RI_b'], g['WF_b'] + g['RF_b'],
                           g['WO_b'] + g['RO_b'], g['WZ_b'] + g['RZ_b']])
    Wcat = np.vstack([Wh, Wz]).astype(f32) * SW              # [1536, 4096]
    Whi8 = _q8(Wcat)
    Wlo8 = _q8(Wcat - Whi8.astype(f32))

    def pack_w(w8, gates):
        arr = w8.reshape(KK, 2, 128, 4, CH, CW)[:, :, :, gates]
        return np.ascontiguousarray(arr.transpose(4, 0, 2, 1, 3, 5))

    whi_l = pack_w(Whi8, list(range(4)))
    wlo_l = pack_w(Wlo8, G3)
    bI, bF, bO, bZ = bias.reshape(4, DFF).astype(f32)
    boz_l = np.ascontiguousarray(
        (np.stack([bO, bZ]) * (SX * SW)).astype(f16)
        .reshape(2, CH, CW).transpose(1, 0, 2))[None]
    mi_shift = (bF - bI)[None, :]

    in_maps = []
    for c in range(NCORES):
        sl = slice(c * BL, (c + 1) * BL)
        Hi_c = g['Hi'][sl].reshape(TOK, DFF)
        Zi_c = g['Zi'][sl].reshape(TOK, D)
        m_c = g['m'][sl].reshape(TOK, 1).astype(f32)
        X = np.concatenate([Hi_c, Zi_c], axis=1).astype(f32) * SX
        XhiT = _q8(X).T                                      # [1536, TOK]
        XloT = _q8(X.T - XhiT.astype(f32))
        mpk = np.concatenate([m_c, 1.0 - m_c, 0.5 * m_c],
                             axis=1).astype(f32).reshape(NT, 128, 3)
        in_maps.append({
            "xhiT": np.ascontiguousarray(XhiT).reshape(KK, 2, 128, TOK),
            "xloT": np.ascontiguousarray(XloT).reshape(KK, 2, 128, TOK),
            "whi": whi_l,
            "wlo": wlo_l,
            "boz": boz_l,
            "mi": (g['Mi'][sl].reshape(TOK, DFF) + mi_shift).astype(f16),
            "ci": g['Ci'][sl].reshape(TOK, DFF).astype(f16),
            "ni": g['Ni'][sl].reshape(TOK, DFF).astype(f16),
            "hiom": ((1.0 - m_c) * Hi_c).astype(f16),
            "mpk": mpk,
        })
    return in_maps, bI


def _gather(results, bI):
    def cat(name):
        full = np.concatenate(
            [results[c][name].astype(np.float32).reshape(BL, P, DFF)
             for c in range(NCORES)], axis=0)
        return np.ascontiguousarray(full, dtype=np.float32)
    mt = cat("mt")
    mt += bI.reshape(1, 1, DFF)
    return cat("ct"), mt, cat("ht"), cat("nt")


def kernel(**inputs):
    nc = _build(repeat=1)
    in_maps, bI = _prep_inputs(inputs)
    res = bass_utils.run_bass_kernel_spmd(nc, in_maps,
                                          core_ids=list(range(NCORES)))
    return _gather(res.results, bI)


# revision 11
# speedup vs baseline: 1.0373x; 1.0373x over previous
"""CustomLSTMCell kernel for Trainium2, data-parallel over batch on 8 cores.

Math (per token, elementwise over dff except the GEMM):
    gates = [Hi|Zi] @ [Wh;Wz] + bias         # [tok, 4*dff], gate order I|F|O|Z
    A = F~ + Mi;  M_t = A - min(A - I~, 0)   # = max(A, I~)
    F_t = exp(min(A - I~, 0));  I_t = exp(min(I~ - A, 0))
    O_t = 0.5*(1 + tanh(O~/2));  Z_t = tanh(Z~)
    N_t = F_t*Ni + I_t
    C_t = (Ci*F_t + Z_t*I_t)*m + (1-m)*Ci
    H_t = O_t*(C_t/N_t)*m + (1-m)*Hi

This workload is HBM-bandwidth-bound on the shared 8-core device, so the
design minimizes bytes (50 MiB/core/exec vs 86 for an fp32-elementwise
version) while staying on the fp16 PE fast path (fp8 DoubleRow measured
slower per MAC-column than modeled on this stack):
- fp16 GEMM, tokens on partitions, activations stationary, weights moving,
  fp32 PSUM accumulate; no PE bias seeds (O/Z biases added by DVE at PSUM
  readout from partition-broadcast tiles; F bias folded into Mi on the host;
  I bias cancels in F_t/I_t and is added to M_t on the host after gather).
- fp16 elementwise in SBUF (DVE 2x/4x perf modes; N_t reciprocal in fp32).
- Packed DMA: the 4 elementwise inputs and 4 outputs travel as single
  [tok, chunk, 4, 512] tensors (4 KiB contiguous runs, 1 descriptor each
  per tile instead of 4); outputs are fp16, upcast on the host.
- Full next-chunk weight double-buffering (wpool 2*KT) to avoid a PE bubble
  at the column-chunk boundary.
Engine split keeps DVE/Act/GPSIMD each under the 10.2 us/tile PE time.
"""

import os
import numpy as np

import concourse.bass as bass
import concourse.tile as tile
import concourse.bass_utils as bass_utils
from concourse import bacc, mybir
from concourse.bass import ts, ds

B, P, D, DFF = 256, 64, 512, 1024
NCORES = 8
BL = B // NCORES
TOK = BL * P              # 2048
NT = TOK // 128           # 16
KH = DFF // 128           # 8
KZ = D // 128             # 4
KT = KH + KZ              # 12
CH = 2
CW = 512

F32 = mybir.dt.float32
F16 = mybir.dt.float16
AF = mybir.ActivationFunctionType
OP = mybir.AluOpType

_CACHE = {}


def _build(repeat: int = 1):
    if repeat in _CACHE:
        return _CACHE[repeat]

    nc = bacc.Bacc("TRN2", target_bir_lowering=False, debug=False,
                   num_devices=NCORES)

    xT = nc.dram_tensor("xT", [KT, 128, TOK], F16, kind="ExternalInput").ap()
    w = nc.dram_tensor("w", [CH, KT, 128, 4, CW], F16, kind="ExternalInput").ap()
    boz = nc.dram_tensor("boz", [2, CH, CW], F32, kind="ExternalInput").ap()
    # packed elementwise inputs: [tok, chunk, (mi|ci|ni|ho), cw] - 4KiB runs
    elin = nc.dram_tensor("elin", [TOK, CH, 4, CW], F16,
                          kind="ExternalInput").ap()
    mpk = nc.dram_tensor("mpk", [NT, 128, 3], F32, kind="ExternalInput").ap()

    # packed outputs: [tok, chunk, (mt|nt|ct|ht), cw]
    out = nc.dram_tensor("out", [TOK, CH, 4, CW], F16,
                         kind="ExternalOutput").ap()

    with tile.TileContext(nc) as tc:
        with (
            tc.tile_pool(name="singles", bufs=1) as singles,
            tc.tile_pool(name="wpool", bufs=2 * KT) as wpool,
            tc.tile_pool(name="inpool", bufs=2) as inpool,
            tc.tile_pool(name="tmpA", bufs=1) as tmpA,
            tc.tile_pool(name="tmpB", bufs=2) as tmpB,
            tc.tile_pool(name="outp", bufs=2) as outp,
            tc.tile_pool(name="ps", bufs=8, space="PSUM") as pspool,
        ):
            xT_sb = singles.tile([128, KT, TOK], F16)
            for k in range(KT):
                nc.sync.dma_start(out=xT_sb[:, k], in_=xT[k])
            mpk_sb = singles.tile([128, NT, 3], F32)
            nc.sync.dma_start(out=mpk_sb, in_=mpk.rearrange("t p c -> p t c"))
            # partition-broadcast O/Z bias tiles [128, CH, CW]
            bb_sb = singles.tile([128, 2, CH, CW], F32)
            for gi in range(2):
                for cj in range(CH):
                    bsl = boz[gi, cj]
                    bcast = bass.AP(tensor=bsl.tensor, offset=bsl.offset,
                                    ap=[[0, 128]] + list(bsl.ap))
                    nc.gpsimd.dma_start(out=bb_sb[:, gi, cj], in_=bcast)

            for _ in range(repeat):
                for c in range(CH):
                    wk = []
                    for k in range(KT):
                        wt = wpool.tile([128, 4, CW], F16, tag="wk")
                        nc.sync.dma_start(out=wt, in_=w[c, k])
                        wk.append(wt)
                    for t in range(NT):
                        rows = ts(t, 128)
                        cols = ds(c * CW, CW)
                        ein = inpool.tile([128, 4, CW], F16, tag="ein")
                        nc.sync.dma_start(out=ein, in_=elin[rows, c])
                        mi_t, ci_t, ni_t, ho_t = (ein[:, 0], ein[:, 1],
                                                  ein[:, 2], ein[:, 3])
                        m_ap = mpk_sb[:, t, 0:1]
                        om_ap = mpk_sb[:, t, 1:2]
                        hm_ap = mpk_sb[:, t, 2:3]

                        ps = [pspool.tile([128, CW], F32, tag="ps",
                                          name=f"ps{g}") for g in range(4)]
                        for k in range(KT):
                            lhsT = xT_sb[:, k, rows]
                            for g in range(4):
                                nc.tensor.matmul(ps[g], lhsT, wk[k][:, g],
                                                 start=(k == 0),
                                                 stop=(k == KT - 1))

                        psI, psF, psO, psZ = ps
                        tmpI = tmpB.tile([128, CW], F16, tag="tmpI")
                        nc.scalar.activation(tmpI, psI, AF.Copy)
                        A = tmpA.tile([128, CW], F16, tag="A")
                        nc.vector.tensor_add(A, psF, mi_t)
                        tO = tmpA.tile([128, CW], F32, tag="tO")
                        nc.vector.tensor_add(tO, psO, bb_sb[:, 0, c])
                        th = tmpB.tile([128, CW], F16, tag="th")
                        nc.scalar.activation(th, tO, AF.Tanh, scale=0.5)
                        tZ = tmpA.tile([128, CW], F32, tag="tZ")
                        nc.vector.tensor_add(tZ, psZ, bb_sb[:, 1, c])
                        Zt = tmpB.tile([128, CW], F16, tag="Zt")
                        nc.scalar.activation(Zt, tZ, AF.Tanh)

                        Dd = tmpA.tile([128, CW], F16, tag="Dd")
                        nc.vector.tensor_sub(Dd, A, tmpI)
                        p_ = tmpA.tile([128, CW], F16, tag="p")
                        nc.vector.tensor_scalar_min(p_, Dd, 0.0)
                        pn = tmpA.tile([128, CW], F16, tag="pn")
                        nc.vector.tensor_scalar(pn, Dd, -1.0, 0.0, OP.mult,
                                                OP.min)
                        eout = outp.tile([128, 4, CW], F16, tag="eout")
                        Mt, Nt, Ct, Ht = (eout[:, 0], eout[:, 1],
                                          eout[:, 2], eout[:, 3])
                        nc.vector.tensor_sub(Mt, A, p_)
                        Ft = tmpB.tile([128, CW], F16, tag="Ft")
                        nc.scalar.activation(Ft, p_, AF.Exp)
                        It = tmpB.tile([128, CW], F16, tag="It")
                        nc.scalar.activation(It, pn, AF.Exp)

                        FN = tmpA.tile([128, CW], F16, tag="FN")
                        nc.gpsimd.tensor_mul(FN, Ft, ni_t)
                        NtF = tmpA.tile([128, CW], F32, tag="NtF")
                        nc.gpsimd.tensor_add(NtF, FN, It)
                        nc.scalar.activation(Nt, NtF, AF.Copy)
                        rec = tmpB.tile([128, CW], F32, tag="rec")
                        nc.vector.reciprocal_approx_fast(rec, NtF)

                        mF = tmpA.tile([128, CW], F16, tag="mF")
                        nc.vector.tensor_scalar(mF, Ft, m_ap, om_ap, OP.mult,
                                                OP.add)
                        p1 = tmpA.tile([128, CW], F16, tag="p1")
                        nc.gpsimd.tensor_mul(p1, ci_t, mF)
                        t2 = tmpA.tile([128, CW], F16, tag="t2")
                        nc.gpsimd.tensor_mul(t2, Zt, It)
                        nc.vector.scalar_tensor_tensor(Ct, t2, m_ap, p1,
                                                       OP.mult, OP.add)

                        thp = tmpA.tile([128, CW], F16, tag="thp")
                        nc.vector.tensor_scalar(thp, th, hm_ap, hm_ap,
                                                OP.mult, OP.add)
                        x1 = tmpA.tile([128, CW], F16, tag="x1")
                        nc.vector.tensor_mul(x1, Ct, rec)
                        x2 = tmpA.tile([128, CW], F16, tag="x2")
                        nc.vector.tensor_mul(x2, x1, thp)
                        nc.vector.tensor_add(Ht, x2, ho_t)

                        nc.sync.dma_start(out=out[rows, c], in_=eout)

    nc.compile()
    _CACHE[repeat] = nc
    return nc


def _prep_inputs(inputs):
    f32, f16 = np.float32, np.float16
    g = {k: np.asarray(v) for k, v in inputs.items()}

    Wh = np.concatenate([g['WI_w'], g['WF_w'], g['WO_w'], g['WZ_w']], axis=1)
    Wz = np.concatenate([g['RI_w'], g['RF_w'], g['RO_w'], g['RZ_w']], axis=1)
    bias = np.concatenate([g['WI_b'] + g['RI_b'], g['WF_b'] + g['RF_b'],
                           g['WO_b'] + g['RO_b'], g['WZ_b'] + g['RZ_b']])
    Wcat = np.vstack([Wh, Wz]).astype(f16)                   # [1536, 4096]
    w_l = np.ascontiguousarray(
        Wcat.reshape(KT, 128, 4, CH, CW).transpose(3, 0, 1, 2, 4))
    bI, bF, bO, bZ = bias.reshape(4, DFF).astype(f32)
    boz_l = np.ascontiguousarray(np.stack([bO, bZ]).reshape(2, CH, CW))
    mi_shift = (bF - bI)[None, :]

    in_maps = []
    for c in range(NCORES):
        sl = slice(c * BL, (c + 1) * BL)
        Hi_c = g['Hi'][sl].reshape(TOK, DFF)
        Zi_c = g['Zi'][sl].reshape(TOK, D)
        m_c = g['m'][sl].reshape(TOK, 1).astype(f32)
        X = np.concatenate([Hi_c, Zi_c], axis=1)
        xT = np.ascontiguousarray(X.T).astype(f16).reshape(KT, 128, TOK)
        mpk = np.concatenate([m_c, 1.0 - m_c, 0.5 * m_c],
                             axis=1).astype(f32).reshape(NT, 128, 3)
        elin = np.empty((TOK, CH, 4, CW), f16)
        for j, arr in enumerate([
                g['Mi'][sl].reshape(TOK, DFF) + mi_shift,
                g['Ci'][sl].reshape(TOK, DFF),
                g['Ni'][sl].reshape(TOK, DFF),
                (1.0 - m_c) * Hi_c]):
            elin[:, :, j, :] = arr.astype(f16).reshape(TOK, CH, CW)
        in_maps.append({
            "xT": xT,
            "w": w_l,
            "boz": boz_l,
            "elin": elin,
            "mpk": mpk,
        })
    return in_maps, bI


def _gather(results, bI):
    def cat(j):
        full = np.concatenate(
            [results[c]["out"][:, :, j, :].astype(np.float32)
             .reshape(BL, P, DFF) for c in range(NCORES)], axis=0)
        return np.ascontiguousarray(full, dtype=np.float32)
    mt = cat(0)
    mt += bI.reshape(1, 1, DFF)
    return cat(2), mt, cat(3), cat(1)


def kernel(**inputs):
    nc = _build(repeat=1)
    in_maps, bI = _prep_inputs(inputs)
    res = bass_utils.run_bass_kernel_spmd(nc, in_maps,
                                          core_ids=list(range(NCORES)))
    return _gather(res.results, bI)
